# revision 18
# baseline (speedup 1.0000x reference)
"""Trainium2 Bass kernel for nn_DecoderCell (B=128,N=512,C=4,T=128,D=128,H=8).

Strategy: pure data-parallel over batch B across 8 NeuronCores (16 b/core).
Per batch, attention scores are computed transposed ([n, q] layout, q=(t,c))
with per-head K=16 matmuls packed 4-at-a-time onto PE row groups (4-way
row-tile concurrency); the boolean mask is applied multiplicatively to the
exp'd scores (es *= (1-m)) on VectorE/GpSimd, keeping the TensorE free of
mask matmuls; softmax runs unnormalized (exp on ScalarE, denominators via an
augmented-V ones column). Final stage is bf16: tanh -> mask-add (-1e8) ->
exp/accum -> fused (x*10 - lnZ) tensor_scalar; output written bf16 and
upcast on host. Q_fixed@Wq_fixed is folded on host into a rank-2 matmul.
"""
import numpy as np
import ml_dtypes

D = 128
N = 512
C = 4
T = 128
Q = T * C          # 512 queries per batch, q = t*C + c
H = 8
DH = 16
NB = 16            # batches per core
NCORES = 8
MNEG = -1.0e8      # final-stage mask value (x10 ~= -1e9), bf16

# log2(m) on [1,2), deg-2 LSQ (max err 9.1e-3 in log2)
L2C2 = -0.33688185
L2C1 = 1.99490327
L2C0 = -1.64899236
LN2 = 0.6931471805599453

HA = [0, 1, 2, 3]
HB = [4, 5, 6, 7]
BF = ml_dtypes.bfloat16


def _perm_cols(W, heads):
    """Columns of W[*,128] so head g sits at cols 32g..32g+15, zeros after."""
    out = np.zeros_like(W)
    for g, h in enumerate(heads):
        out[:, 32 * g:32 * g + 16] = W[:, 16 * h:16 * h + 16]
    return out


def _perm_rows(W, heads):
    out = np.zeros_like(W)
    for g, h in enumerate(heads):
        out[32 * g:32 * g + 16, :] = W[16 * h:16 * h + 16, :]
    return out


def _host_prep(inputs):
    """Full-input numpy prep -> per-core input dicts."""
    ne = np.ascontiguousarray(inputs["node_embeddings"], np.float32)
    ge = np.ascontiguousarray(inputs["graph_embedding"], np.float32)
    sc = np.ascontiguousarray(inputs["step_context"], np.float32)
    mask = np.asarray(inputs["mask"])
    B = ne.shape[0]

    net = ne.transpose(0, 2, 1).astype(BF)                        # [B,D,N]
    scq = sc[:, :, :, 0, :].transpose(1, 3, 0, 2).reshape(B, D + 1, Q)
    scm = scq[:, :D, :].astype(BF)
    # [scl; ones] rank-2 rhs rows
    scl2 = np.concatenate(
        [scq[:, D:, :], np.ones((B, 1, Q), np.float32)], axis=1).astype(BF)
    m4 = mask[:, :, :, 0, :]                                      # [T,B,C,N] bool
    # keep-mask (1-m), transposed [B,N,Q] -> [B,128,4(j),Q] bf16
    mt = m4.transpose(1, 3, 0, 2).reshape(B, N, Q)
    mkeep = ((1.0 - mt.reshape(B, 4, 128, Q).transpose(0, 2, 1, 3)
              .astype(np.float32))).astype(BF)                    # [B,128,4,Q]
    # natural mask [B,Q,N] -> [B,128,4(i),N] bf16, value MNEG
    mn = m4.transpose(1, 0, 2, 3).reshape(B, Q, N)
    mknegn = (mn.reshape(B, 4, 128, N).transpose(0, 2, 1, 3).astype(np.float32)
              * np.float32(MNEG)).astype(BF)                      # [B,128,4,N]

    s = np.float32(1.0 / np.sqrt(DH))
    Wk1 = np.asarray(inputs["Wk1"], np.float32)
    Wqs = np.asarray(inputs["Wq_step"], np.float32) * s
    Wqf = np.asarray(inputs["Wq_fixed"], np.float32) * s
    wqsa, wqsb = _perm_cols(Wqs, HA), _perm_cols(Wqs, HB)
    wqfa, wqfb = _perm_cols(Wqf, HA), _perm_cols(Wqf, HB)
    # host-side Q_fixed projections (tiny): [B, 128] packed per pass
    qfa = ge @ wqfa
    qfb = ge @ wqfb
    # rank-2 lhsT per batch: rows [wq_step_last; qfix]
    qsl2a = np.stack([np.broadcast_to(wqsa[D], (B, 128)), qfa],
                     axis=1).astype(BF)                           # [B,2,128]
    qsl2b = np.stack([np.broadcast_to(wqsb[D], (B, 128)), qfb],
                     axis=1).astype(BF)
    bfw = lambda x: np.ascontiguousarray(x).astype(BF)
    # e4 padded to [128,128]: pass-A selector rows 0-3, pass-B rows 32-35
    e4p = np.zeros((128, 128), np.float32)
    for g in range(4):
        e4p[g, 32 * g:32 * g + 16] = 1.0
        e4p[32 + g, 32 * g:32 * g + 16] = 1.0
    weights = {
        "wk1a": bfw(_perm_cols(Wk1, HA)), "wk1b": bfw(_perm_cols(Wk1, HB)),
        "wqsam": bfw(wqsa[:D]), "wqsbm": bfw(wqsb[:D]),
        "wv": bfw(inputs["Wv"]),
        "wk2s": bfw(np.asarray(inputs["Wk2"], np.float32)
                    / np.float32(np.sqrt(D))),
        "wouta": bfw(_perm_rows(np.asarray(inputs["Wout"], np.float32), HA)),
        "woutb": bfw(_perm_rows(np.asarray(inputs["Wout"], np.float32), HB)),
        "e4p": e4p.astype(BF),
        # g16[32g+16, g] = 1 (Z row gather)
        "g16": np.stack([
            (np.arange(128) == 32 * g + 16).astype(np.float32)
            for g in range(4)], axis=1).astype(BF),
        # p4sel[q', t'] = 1 iff q'//4 == t'  (c-sum per step)
        "p4sel": np.stack([
            ((np.arange(128) // 4) == tp).astype(np.float32)
            for tp in range(32)], axis=1).astype(BF),
        # p432[t', q'] = 1 iff q'//4 == t'  (lnZ partition broadcast)
        "p432": np.stack([
            ((np.arange(128) // 4) == tp).astype(np.float32)
            for tp in range(32)]).astype(BF),
    }

    core_ins = []
    for ci in range(NCORES):
        b0 = ci * NB
        sl = slice(b0, b0 + NB)
        m = dict(weights)
        m.update({
            "net": np.ascontiguousarray(net[sl]),
            "scm": np.ascontiguousarray(scm[sl]),
            "scl2": np.ascontiguousarray(scl2[sl]),
            "qsl2a": np.ascontiguousarray(qsl2a[sl]),
            "qsl2b": np.ascontiguousarray(qsl2b[sl]),
            "mkeep": np.ascontiguousarray(mkeep[sl]),
            "mknegn": np.ascontiguousarray(mknegn[sl]),
        })
        core_ins.append(m)
    return core_ins


def build_kernel(nb=NB):
    import concourse.bacc as bacc
    import concourse.mybir as mybir
    import concourse.tile as tile

    dt = mybir.dt
    f32, bf16, i32 = dt.float32, dt.bfloat16, dt.int32
    AF = mybir.ActivationFunctionType
    OP = mybir.AluOpType

    nc = bacc.Bacc("TRN2", target_bir_lowering=False, debug=False,
                   num_devices=NCORES)

    din = {}
    def dram(name, shape, dtype, kind="ExternalInput"):
        din[name] = nc.dram_tensor(name, shape, dtype, kind=kind)
        return din[name]

    net = dram("net", [nb, D, N], bf16)
    scm = dram("scm", [nb, D, Q], bf16)
    scl2 = dram("scl2", [nb, 2, Q], bf16)
    qsl2a = dram("qsl2a", [nb, 2, 128], bf16)
    qsl2b = dram("qsl2b", [nb, 2, 128], bf16)
    mkeep = dram("mkeep", [nb, 128, 4, Q], bf16)
    mknegn = dram("mknegn", [nb, 128, 4, N], bf16)
    for w in ("wk1a", "wk1b", "wqsam", "wqsbm", "wv", "wk2s",
              "wouta", "woutb", "e4p", "g16", "p4sel", "p432"):
        shape = ([128, 4] if w == "g16" else
                 ([128, 32] if w == "p4sel" else
                  ([32, 128] if w == "p432" else [128, 128])))
        dram(w, shape, bf16)
    # device layout [q'=(t', c), b, i, n]; host reassembles t = 32*i + t'
    out = dram("out", [128, nb, 4, N], bf16, kind="ExternalOutput")

    with tile.TileContext(nc) as tc:
        from contextlib import ExitStack
        with ExitStack() as ctx:
            wp = ctx.enter_context(tc.tile_pool(name="wp", bufs=1))
            io = ctx.enter_context(tc.tile_pool(name="io", bufs=2))
            wk = ctx.enter_context(tc.tile_pool(name="wk", bufs=2))
            big = ctx.enter_context(tc.tile_pool(name="big", bufs=2))
            sm = ctx.enter_context(tc.tile_pool(name="sm", bufs=2))
            pss = ctx.enter_context(tc.tile_pool(name="pss", bufs=2, space="PSUM"))
            pu = ctx.enter_context(tc.tile_pool(name="pu", bufs=1, space="PSUM"))
            pproj = ctx.enter_context(tc.tile_pool(name="pproj", bufs=2, space="PSUM"))
            pt = ctx.enter_context(tc.tile_pool(name="pt", bufs=1, space="PSUM"))

            # --- static weights/constants to SBUF ---
            W = {}
            for wn in ("wk1a", "wk1b", "wqsam", "wqsbm", "wv", "wk2s",
                       "wouta", "woutb", "e4p", "g16", "p4sel", "p432"):
                t = wp.tile(list(din[wn].shape), din[wn].dtype, tag=f"w_{wn}")
                nc.sync.dma_start(out=t, in_=din[wn][:, :])
                W[wn] = t

            for b in range(nb):
                # ---------- DMA loads ----------
                net_t = io.tile([D, N], bf16, tag="net")
                nc.sync.dma_start(out=net_t, in_=net[b])
                scm_t = io.tile([D, Q], bf16, tag="scm")
                nc.sync.dma_start(out=scm_t, in_=scm[b])
                scl2_t = io.tile([2, Q], bf16, tag="scl2")
                nc.sync.dma_start(out=scl2_t, in_=scl2[b])
                q2a_t = io.tile([2, 128], bf16, tag="qsl2a")
                nc.sync.dma_start(out=q2a_t, in_=qsl2a[b])
                q2b_t = io.tile([2, 128], bf16, tag="qsl2b")
                nc.sync.dma_start(out=q2b_t, in_=qsl2b[b])
                mk_t = io.tile([128, 4, Q], bf16, tag="mkeep")
                nc.sync.dma_start(out=mk_t, in_=mkeep[b])
                mkn_t = io.tile([128, 4, N], bf16, tag="mknegn")
                nc.sync.dma_start(out=mkn_t, in_=mknegn[b])

                # ---------- projections ----------
                def proj_to_sbuf(wtile, rhs, tag):
                    ps = pproj.tile([128, N], f32, tag="proj")
                    nc.tensor.matmul(ps, lhsT=wtile, rhs=rhs)
                    sb = wk.tile([128, N], bf16, tag=tag)
                    nc.vector.tensor_copy(sb, ps)
                    return sb

                k1ta = proj_to_sbuf(W["wk1a"], net_t, "k1ta")
                k1tb = proj_to_sbuf(W["wk1b"], net_t, "k1tb")
                k2t = proj_to_sbuf(W["wk2s"], net_t, "k2t")

                # V natural per n-chunk, packed into V_aug [128, 4(j), 128]
                vauga = wk.tile([128, 4, 128], bf16, tag="vauga")
                vaugb = wk.tile([128, 4, 128], bf16, tag="vaugb")
                for va in (vauga, vaugb):
                    nc.gpsimd.memset(va, 0.0)
                    nc.gpsimd.memset(
                        va.rearrange("p j (g r) -> p j g r", g=4)[:, :, :, 16:17],
                        1.0)
                for j in range(4):
                    pv = pproj.tile([128, N], f32, tag="proj")
                    nc.tensor.matmul(
                        pv[:, :128], lhsT=net_t[:, 128 * j:128 * (j + 1)],
                        rhs=W["wv"])
                    for va, c0 in ((vauga, 0), (vaugb, 64)):
                        nc.vector.tensor_copy(
                            va[:, j, :].rearrange("p (g r) -> p g r", g=4)[:, :, 0:16],
                            pv[:, c0:c0 + 64].rearrange("p (g r) -> p g r", g=4))

                # ---------- Q1T (passes A and B) ----------
                def q1t(wm, ql2, tag):
                    ps = pproj.tile([128, Q], f32, tag="proj")
                    nc.tensor.matmul(ps, lhsT=wm, rhs=scm_t, start=True, stop=False)
                    nc.tensor.matmul(ps, lhsT=ql2, rhs=scl2_t,
                                     start=False, stop=True)
                    sb = wk.tile([128, Q], bf16, tag=tag)
                    nc.vector.tensor_copy(sb, ps)
                    return sb

                q1ta = q1t(W["wqsam"], q2a_t, "q1ta")
                q1tb = q1t(W["wqsbm"], q2b_t, "q1tb")

                # ---------- attention passes ----------
                u2 = wk.tile([128, 2, Q], bf16, tag="u2")
                pzAB = pt.tile([36, Q], f32, tag="t")
                for pi, (k1t, q1t_sb, vaug) in enumerate(
                        ((k1ta, q1ta, vauga), (k1tb, q1tb, vaugb))):
                    psu = pu.tile([128, Q], f32, tag="u")
                    for j in range(4):
                        psA = pss.tile([128, 2, Q], f32, tag="sc")
                        psB = pss.tile([128, 2, Q], f32, tag="sc")
                        # 4 score MMs on distinct row groups (concurrent)
                        for g in range(4):
                            ps2 = psA if g < 2 else psB
                            sl = slice(32 * g, 32 * g + 16)
                            nc.tensor.matmul(
                                ps2[:, g % 2, :],
                                lhsT=k1t[sl, 128 * j:128 * (j + 1)],
                                rhs=q1t_sb[sl, :], start=True, stop=True,
                                tile_position=(32 * g, 0),
                                skip_group_check=True)
                        esA = big.tile([128, 2, Q], bf16, tag="es")
                        esB = big.tile([128, 2, Q], bf16, tag="es")
                        nc.scalar.activation(esA, psA, AF.Exp)
                        nc.scalar.activation(esB, psB, AF.Exp)
                        # mask multiply (1-m), broadcast over the head pair
                        emA = big.tile([128, 2, Q], bf16, tag="esm")
                        emB = big.tile([128, 2, Q], bf16, tag="esm")
                        mkb = (mk_t[:, j, :]
                               .rearrange("p (a q) -> p a q", a=1)
                               .broadcast_to([128, 2, Q]))
                        nc.vector.tensor_tensor(emA, esA, mkb, OP.mult)
                        nc.vector.tensor_tensor(emB, esB, mkb, OP.mult)
                        for g in range(4):
                            em = emA if g < 2 else emB
                            nc.tensor.matmul(
                                psu[32 * g:32 * g + 32, :],
                                lhsT=vaug[:, j, 32 * g:32 * g + 32],
                                rhs=em[:, g % 2, :],
                                start=(j == 0), stop=(j == 3),
                                tile_position=(0, 32 * g),
                                skip_group_check=True)
                    # copy U to sbuf; gather Z rows (pass A -> parts 0-3,
                    # pass B -> parts 32-35 via column tiling)
                    nc.scalar.copy(u2[:, pi, :], psu)
                    nc.tensor.matmul(pzAB[32 * pi:32 * pi + 4, :],
                                     lhsT=W["g16"], rhs=u2[:, pi, :],
                                     tile_position=(0, 32 * pi),
                                     skip_group_check=True)

                zsb = sm.tile([36, Q], f32, tag="zsb")
                nc.vector.tensor_copy(zsb, pzAB)
                rinv = sm.tile([36, Q], f32, tag="rinv")
                nc.vector.reciprocal_approx_fast(out=rinv, in_=zsb)
                rinvb = sm.tile([36, Q], bf16, tag="rinvb")
                nc.gpsimd.tensor_copy(rinvb, rinv)

                un = {}
                pbc = {}
                for pi in range(2):
                    r0 = 32 * pi
                    pool = pt if pi == 0 else pu
                    pbc[pi] = pool.tile([128, Q], f32, name=f"pbc{pi}",
                                        tag=("t" if pi == 0 else "u"))
                    nc.tensor.matmul(pbc[pi], lhsT=W["e4p"][r0:r0 + 4, :],
                                     rhs=rinvb[r0:r0 + 4, :],
                                     tile_position=(r0, 0),
                                     skip_group_check=True)
                for pi in range(2):
                    u_n = wk.tile([128, Q], bf16, tag=f"un{pi}")
                    nc.vector.tensor_tensor(u_n, u2[:, pi, :], pbc[pi], OP.mult)
                    un[pi] = u_n

                # ---------- Q2 and logits ----------
                pq2 = pproj.tile([128, Q], f32, tag="proj")
                nc.tensor.matmul(pq2, lhsT=W["wouta"], rhs=un[0],
                                 start=True, stop=False)
                nc.tensor.matmul(pq2, lhsT=W["woutb"], rhs=un[1],
                                 start=False, stop=True)
                q2t = wk.tile([128, Q], bf16, tag="q2t")
                nc.scalar.copy(q2t, pq2)

                # ---------- final stage (bf16) ----------
                s1 = big.tile([128, 4, N], bf16, tag="s1")
                for ii in range(2):
                    pl = pss.tile([128, 2, N], f32, tag="sc")
                    for i2 in range(2):
                        i = 2 * ii + i2
                        nc.tensor.matmul(
                            pl[:, i2, :],
                            lhsT=q2t[:, 128 * i:128 * (i + 1)], rhs=k2t)
                    th = big.tile([128, 2, N], bf16, tag="th")
                    nc.scalar.activation(th, pl, AF.Tanh)
                    nc.gpsimd.tensor_tensor(
                        s1[:, 2 * ii:2 * ii + 2, :], th,
                        mkn_t[:, 2 * ii:2 * ii + 2, :], OP.add)

                sacc = sm.tile([128, 4], f32, tag="sacc")
                es2 = big.tile([128, 4, N], bf16, tag="es2")
                for i in range(4):
                    nc.scalar.activation(es2[:, i, :], s1[:, i, :], AF.Exp,
                                         scale=10.0, accum_out=sacc[:, i:i + 1])
                saccb = sm.tile([128, 4], bf16, tag="saccb")
                nc.vector.tensor_copy(saccb, sacc)
                pz2 = pt.tile([32, 4], f32, tag="t")
                nc.tensor.matmul(pz2, lhsT=W["p4sel"], rhs=saccb)
                z2 = sm.tile([32, 4], f32, tag="z2")
                nc.vector.tensor_copy(z2, pz2)
                # lnZ = (c2*m^2 + c1*m)*ln2 + (e - 127 + c0)*ln2
                zi = z2.bitcast(i32)
                ei = sm.tile([32, 4], i32, tag="ei")
                nc.vector.tensor_scalar(ei, zi, 23, None,
                                        OP.logical_shift_right)
                ef = sm.tile([32, 4], f32, tag="ef")
                nc.vector.tensor_copy(ef, ei)
                mi = sm.tile([32, 4], i32, tag="mi")
                nc.vector.tensor_scalar(mi, zi, 0x7FFFFF, 0x3F800000,
                                        OP.bitwise_and, OP.bitwise_or)
                mf = mi.bitcast(f32)
                acc = sm.tile([32, 4], f32, tag="lnacc")
                nc.vector.tensor_scalar(acc, mf, L2C2, L2C1, OP.mult, OP.add)
                nc.vector.tensor_tensor(acc, acc, mf, OP.mult)
                nc.vector.tensor_scalar(ef, ef, LN2, (L2C0 - 127.0) * LN2,
                                        OP.mult, OP.add)
                lnzb = sm.tile([32, 4], bf16, tag="lnzb")
                nc.vector.scalar_tensor_tensor(lnzb, acc, LN2, ef,
                                               OP.mult, OP.add)
                pbias = pt.tile([128, 4], f32, tag="t")
                nc.tensor.matmul(pbias, lhsT=W["p432"], rhs=lnzb)
                bias = sm.tile([128, 4], f32, tag="bias")
                nc.vector.tensor_copy(bias, pbias)

                out_sb = big.tile([128, 4, N], bf16, tag="outsb")
                for i in range(4):
                    nc.gpsimd.tensor_scalar(
                        out_sb[:, i, :], s1[:, i, :], 10.0,
                        bias[:, i:i + 1], OP.mult, OP.subtract)
                nc.sync.dma_start(out=out[:, b, :, :], in_=out_sb)

    nc.compile()
    return nc


_CACHED = None


def _get_nc():
    global _CACHED
    if _CACHED is None:
        _CACHED = build_kernel()
    return _CACHED


def kernel(**inputs):
    from concourse.bass_utils import run_bass_kernel_spmd

    core_ins = _host_prep(inputs)
    nc = _get_nc()
    res = run_bass_kernel_spmd(nc, core_ins, core_ids=list(range(NCORES)))
    outs = [_unscramble(r["out"]) for r in res.results]   # each [T, NB, 2048]
    return np.concatenate(outs, axis=1)                   # [T, B, 2048]


def _unscramble(dev):
    """Device [128 q'=(t',c), nb, 4 i, 512 n] -> [T, nb, C*N] with t=32i+t'."""
    nb = dev.shape[1]
    return (dev.astype(np.float32)
            .reshape(32, C, nb, 4, N)
            .transpose(3, 0, 2, 1, 4)
            .reshape(T, nb, C * N))


# revision 20
# speedup vs baseline: 1.1423x; 1.1423x over previous
"""Trainium2 Bass kernel for nn_DecoderCell (B=128,N=512,C=4,T=128,D=128,H=8).

Strategy: pure data-parallel over batch B across 8 NeuronCores (16 b/core).
Per batch, attention scores are computed transposed ([n, q] layout, q=(t,c))
with per-head K=16 matmuls packed 4-at-a-time onto PE row groups (4-way
row-tile concurrency); the boolean mask is applied multiplicatively to the
exp'd scores (es *= (1-m)) on VectorE/GpSimd, keeping the TensorE free of
mask matmuls; softmax runs unnormalized (exp on ScalarE, denominators via an
augmented-V ones column). Final stage is bf16: tanh -> mask-add (-1e8) ->
exp/accum -> fused (x*10 - lnZ) tensor_scalar; output written bf16 and
upcast on host. Q_fixed@Wq_fixed is folded on host into a rank-2 matmul.
"""
import numpy as np
import ml_dtypes

D = 128
N = 512
C = 4
T = 128
Q = T * C          # 512 queries per batch, q = t*C + c
H = 8
DH = 16
NB = 16            # batches per core
NCORES = 8
MNEG = -1.0e8      # final-stage mask value (x10 ~= -1e9), bf16

# log2(m) on [1,2), deg-2 LSQ (max err 9.1e-3 in log2)
L2C2 = -0.33688185
L2C1 = 1.99490327
L2C0 = -1.64899236
LN2 = 0.6931471805599453

HA = [0, 1, 2, 3]
HB = [4, 5, 6, 7]
BF = ml_dtypes.bfloat16


def _perm_cols(W, heads):
    """Columns of W[*,128] so head g sits at cols 32g..32g+15, zeros after."""
    out = np.zeros_like(W)
    for g, h in enumerate(heads):
        out[:, 32 * g:32 * g + 16] = W[:, 16 * h:16 * h + 16]
    return out


def _perm_rows(W, heads):
    out = np.zeros_like(W)
    for g, h in enumerate(heads):
        out[32 * g:32 * g + 16, :] = W[16 * h:16 * h + 16, :]
    return out


def _host_prep(inputs):
    """Full-input numpy prep -> per-core input dicts."""
    ne = np.ascontiguousarray(inputs["node_embeddings"], np.float32)
    ge = np.ascontiguousarray(inputs["graph_embedding"], np.float32)
    sc = np.ascontiguousarray(inputs["step_context"], np.float32)
    mask = np.asarray(inputs["mask"])
    B = ne.shape[0]

    net = ne.transpose(0, 2, 1).astype(BF)                        # [B,D,N]
    scq = sc[:, :, :, 0, :].transpose(1, 3, 0, 2).reshape(B, D + 1, Q)
    scm = scq[:, :D, :].astype(BF)
    # [scl; ones] rank-2 rhs rows
    scl2 = np.concatenate(
        [scq[:, D:, :], np.ones((B, 1, Q), np.float32)], axis=1).astype(BF)
    m4 = mask[:, :, :, 0, :]                                      # [T,B,C,N] bool
    # keep-mask (1-m), transposed [B,N,Q] -> [B,128,4(j),Q] bf16
    mt = m4.transpose(1, 3, 0, 2).reshape(B, N, Q)
    mkeep = ((1.0 - mt.reshape(B, 4, 128, Q).transpose(0, 2, 1, 3)
              .astype(np.float32))).astype(BF)                    # [B,128,4,Q]
    # natural mask [B,Q,N] -> [B,128,4(i),N] bf16, value MNEG
    mn = m4.transpose(1, 0, 2, 3).reshape(B, Q, N)
    mknegn = (mn.reshape(B, 4, 128, N).transpose(0, 2, 1, 3).astype(np.float32)
              * np.float32(MNEG)).astype(BF)                      # [B,128,4,N]

    s = np.float32(1.0 / np.sqrt(DH))
    Wk1 = np.asarray(inputs["Wk1"], np.float32)
    Wqs = np.asarray(inputs["Wq_step"], np.float32) * s
    Wqf = np.asarray(inputs["Wq_fixed"], np.float32) * s
    wqsa, wqsb = _perm_cols(Wqs, HA), _perm_cols(Wqs, HB)
    wqfa, wqfb = _perm_cols(Wqf, HA), _perm_cols(Wqf, HB)
    # host-side Q_fixed projections (tiny): [B, 128] packed per pass
    qfa = ge @ wqfa
    qfb = ge @ wqfb
    # rank-2 lhsT per batch: rows [wq_step_last; qfix]
    qsl2a = np.stack([np.broadcast_to(wqsa[D], (B, 128)), qfa],
                     axis=1).astype(BF)                           # [B,2,128]
    qsl2b = np.stack([np.broadcast_to(wqsb[D], (B, 128)), qfb],
                     axis=1).astype(BF)
    bfw = lambda x: np.ascontiguousarray(x).astype(BF)
    # e4 padded to [128,128]: pass-A selector rows 0-3, pass-B rows 32-35
    e4p = np.zeros((128, 128), np.float32)
    for g in range(4):
        e4p[g, 32 * g:32 * g + 16] = 1.0
        e4p[32 + g, 32 * g:32 * g + 16] = 1.0
    weights = {
        "wk1a": bfw(_perm_cols(Wk1, HA)), "wk1b": bfw(_perm_cols(Wk1, HB)),
        "wqsam": bfw(wqsa[:D]), "wqsbm": bfw(wqsb[:D]),
        "wv": bfw(inputs["Wv"]),
        "wk2s": bfw(np.asarray(inputs["Wk2"], np.float32)
                    / np.float32(np.sqrt(D))),
        "wouta": bfw(_perm_rows(np.asarray(inputs["Wout"], np.float32), HA)),
        "woutb": bfw(_perm_rows(np.asarray(inputs["Wout"], np.float32), HB)),
        "e4p": e4p.astype(BF),
        # g16[32g+16, g] = 1 (Z row gather)
        "g16": np.stack([
            (np.arange(128) == 32 * g + 16).astype(np.float32)
            for g in range(4)], axis=1).astype(BF),
        # p4sel[q', t'] = 1 iff q'//4 == t'  (c-sum per step)
        "p4sel": np.stack([
            ((np.arange(128) // 4) == tp).astype(np.float32)
            for tp in range(32)], axis=1).astype(BF),
        # p432[t', q'] = 1 iff q'//4 == t'  (lnZ partition broadcast)
        "p432": np.stack([
            ((np.arange(128) // 4) == tp).astype(np.float32)
            for tp in range(32)]).astype(BF),
    }

    core_ins = []
    for ci in range(NCORES):
        b0 = ci * NB
        sl = slice(b0, b0 + NB)
        m = dict(weights)
        m.update({
            "net": np.ascontiguousarray(net[sl]),
            "scm": np.ascontiguousarray(scm[sl]),
            "scl2": np.ascontiguousarray(scl2[sl]),
            "qsl2a": np.ascontiguousarray(qsl2a[sl]),
            "qsl2b": np.ascontiguousarray(qsl2b[sl]),
            "mkeep": np.ascontiguousarray(mkeep[sl]),
            "mknegn": np.ascontiguousarray(mknegn[sl]),
        })
        core_ins.append(m)
    return core_ins


def build_kernel(nb=NB):
    import concourse.bacc as bacc
    import concourse.mybir as mybir
    import concourse.tile as tile

    dt = mybir.dt
    f32, bf16, i32 = dt.float32, dt.bfloat16, dt.int32
    AF = mybir.ActivationFunctionType
    OP = mybir.AluOpType

    nc = bacc.Bacc("TRN2", target_bir_lowering=False, debug=False,
                   num_devices=NCORES)

    din = {}
    def dram(name, shape, dtype, kind="ExternalInput"):
        din[name] = nc.dram_tensor(name, shape, dtype, kind=kind)
        return din[name]

    net = dram("net", [nb, D, N], bf16)
    scm = dram("scm", [nb, D, Q], bf16)
    scl2 = dram("scl2", [nb, 2, Q], bf16)
    qsl2a = dram("qsl2a", [nb, 2, 128], bf16)
    qsl2b = dram("qsl2b", [nb, 2, 128], bf16)
    mkeep = dram("mkeep", [nb, 128, 4, Q], bf16)
    mknegn = dram("mknegn", [nb, 128, 4, N], bf16)
    for w in ("wk1a", "wk1b", "wqsam", "wqsbm", "wv", "wk2s",
              "wouta", "woutb", "e4p", "g16", "p4sel", "p432"):
        shape = ([128, 4] if w == "g16" else
                 ([128, 32] if w == "p4sel" else
                  ([32, 128] if w == "p432" else [128, 128])))
        dram(w, shape, bf16)
    # device layout [q'=(t', c), b, i, n]; host reassembles t = 32*i + t'
    out = dram("out", [128, nb, 4, N], bf16, kind="ExternalOutput")

    with tile.TileContext(nc) as tc:
        from contextlib import ExitStack
        with ExitStack() as ctx:
            wp = ctx.enter_context(tc.tile_pool(name="wp", bufs=1))
            io = ctx.enter_context(tc.tile_pool(name="io", bufs=2))
            wk = ctx.enter_context(tc.tile_pool(name="wk", bufs=2))
            big = ctx.enter_context(tc.tile_pool(name="big", bufs=2))
            sm = ctx.enter_context(tc.tile_pool(name="sm", bufs=2))
            pss = ctx.enter_context(tc.tile_pool(name="pss", bufs=2, space="PSUM"))
            pu = ctx.enter_context(tc.tile_pool(name="pu", bufs=1, space="PSUM"))
            pproj = ctx.enter_context(tc.tile_pool(name="pproj", bufs=2, space="PSUM"))
            pt = ctx.enter_context(tc.tile_pool(name="pt", bufs=1, space="PSUM"))

            # --- static weights/constants to SBUF ---
            W = {}
            for wn in ("wk1a", "wk1b", "wqsam", "wqsbm", "wv", "wk2s",
                       "wouta", "woutb", "e4p", "g16", "p4sel", "p432"):
                t = wp.tile(list(din[wn].shape), din[wn].dtype, tag=f"w_{wn}")
                nc.sync.dma_start(out=t, in_=din[wn][:, :])
                W[wn] = t

            for b in range(nb):
                # ---------- DMA loads ----------
                net_t = io.tile([D, N], bf16, tag="net")
                nc.sync.dma_start(out=net_t, in_=net[b])
                scm_t = io.tile([D, Q], bf16, tag="scm")
                nc.sync.dma_start(out=scm_t, in_=scm[b])
                scl2_t = io.tile([2, Q], bf16, tag="scl2")
                nc.sync.dma_start(out=scl2_t, in_=scl2[b])
                q2a_t = io.tile([2, 128], bf16, tag="qsl2a")
                nc.sync.dma_start(out=q2a_t, in_=qsl2a[b])
                q2b_t = io.tile([2, 128], bf16, tag="qsl2b")
                nc.sync.dma_start(out=q2b_t, in_=qsl2b[b])
                mk_t = io.tile([128, 4, Q], bf16, tag="mkeep")
                nc.sync.dma_start(out=mk_t, in_=mkeep[b])
                mkn_t = io.tile([128, 4, N], bf16, tag="mknegn")
                nc.sync.dma_start(out=mkn_t, in_=mknegn[b])

                # ---------- projections ----------
                def proj_to_sbuf(wtile, rhs, tag):
                    ps = pproj.tile([128, N], f32, tag="proj")
                    nc.tensor.matmul(ps, lhsT=wtile, rhs=rhs)
                    sb = wk.tile([128, N], bf16, tag=tag)
                    nc.vector.tensor_copy(sb, ps)
                    return sb

                k1ta = proj_to_sbuf(W["wk1a"], net_t, "k1ta")
                k1tb = proj_to_sbuf(W["wk1b"], net_t, "k1tb")
                k2t = proj_to_sbuf(W["wk2s"], net_t, "k2t")

                # V natural per n-chunk, packed into V_aug [128, 4(j), 128]
                vauga = wk.tile([128, 4, 128], bf16, tag="vauga")
                vaugb = wk.tile([128, 4, 128], bf16, tag="vaugb")
                for va in (vauga, vaugb):
                    nc.gpsimd.memset(va, 0.0)
                    nc.gpsimd.memset(
                        va.rearrange("p j (g r) -> p j g r", g=4)[:, :, :, 16:17],
                        1.0)
                for j in range(4):
                    pv = pproj.tile([128, N], f32, tag="proj")
                    nc.tensor.matmul(
                        pv[:, :128], lhsT=net_t[:, 128 * j:128 * (j + 1)],
                        rhs=W["wv"])
                    for va, c0 in ((vauga, 0), (vaugb, 64)):
                        nc.vector.tensor_copy(
                            va[:, j, :].rearrange("p (g r) -> p g r", g=4)[:, :, 0:16],
                            pv[:, c0:c0 + 64].rearrange("p (g r) -> p g r", g=4))

                # ---------- Q1T (passes A and B) ----------
                def q1t(wm, ql2, tag):
                    ps = pproj.tile([128, Q], f32, tag="proj")
                    nc.tensor.matmul(ps, lhsT=wm, rhs=scm_t, start=True, stop=False)
                    nc.tensor.matmul(ps, lhsT=ql2, rhs=scl2_t,
                                     start=False, stop=True)
                    sb = wk.tile([128, Q], bf16, tag=tag)
                    nc.vector.tensor_copy(sb, ps)
                    return sb

                q1ta = q1t(W["wqsam"], q2a_t, "q1ta")
                q1tb = q1t(W["wqsbm"], q2b_t, "q1tb")

                # ---------- attention passes ----------
                u2 = wk.tile([128, 2, Q], bf16, tag="u2")
                pzAB = pt.tile([36, Q], f32, tag="t")
                for pi, (k1t, q1t_sb, vaug) in enumerate(
                        ((k1ta, q1ta, vauga), (k1tb, q1tb, vaugb))):
                    psu = pu.tile([128, Q], f32, tag="u")
                    for j in range(4):
                        psA = pss.tile([128, 2, Q], f32, tag="sc")
                        psB = pss.tile([128, 2, Q], f32, tag="sc")
                        # 4 score MMs on distinct row groups (concurrent)
                        for g in range(4):
                            ps2 = psA if g < 2 else psB
                            sl = slice(32 * g, 32 * g + 16)
                            nc.tensor.matmul(
                                ps2[:, g % 2, :],
                                lhsT=k1t[sl, 128 * j:128 * (j + 1)],
                                rhs=q1t_sb[sl, :], start=True, stop=True,
                                tile_position=(32 * g, 0),
                                skip_group_check=True)
                        esA = big.tile([128, 2, Q], bf16, tag="es")
                        esB = big.tile([128, 2, Q], bf16, tag="es")
                        nc.scalar.activation(esA, psA, AF.Exp)
                        nc.scalar.activation(esB, psB, AF.Exp)
                        # mask multiply (1-m): DVE(5)/GpSimd(3) split per pass
                        emA = big.tile([128, 2, Q], bf16, tag="esm")
                        emB = big.tile([128, 2, Q], bf16, tag="esm")
                        engA = nc.vector
                        engB = nc.gpsimd if j % 3 == 0 else nc.vector
                        for gg in range(2):
                            engA.tensor_tensor(
                                emA[:, gg, :], esA[:, gg, :], mk_t[:, j, :],
                                OP.mult)
                            engB.tensor_tensor(
                                emB[:, gg, :], esB[:, gg, :], mk_t[:, j, :],
                                OP.mult)
                        for g in range(4):
                            em = emA if g < 2 else emB
                            nc.tensor.matmul(
                                psu[32 * g:32 * g + 32, :],
                                lhsT=vaug[:, j, 32 * g:32 * g + 32],
                                rhs=em[:, g % 2, :],
                                start=(j == 0), stop=(j == 3),
                                tile_position=(0, 32 * g),
                                skip_group_check=True)
                    # copy U to sbuf; gather Z rows (pass A -> parts 0-3,
                    # pass B -> parts 32-35 via column tiling)
                    nc.scalar.copy(u2[:, pi, :], psu)
                    nc.tensor.matmul(pzAB[32 * pi:32 * pi + 4, :],
                                     lhsT=W["g16"], rhs=u2[:, pi, :],
                                     tile_position=(0, 32 * pi),
                                     skip_group_check=True)

                zsb = sm.tile([36, Q], f32, tag="zsb")
                nc.vector.tensor_copy(zsb, pzAB)
                rinv = sm.tile([36, Q], f32, tag="rinv")
                nc.vector.reciprocal_approx_fast(out=rinv, in_=zsb)
                rinvb = sm.tile([36, Q], bf16, tag="rinvb")
                nc.gpsimd.tensor_copy(rinvb, rinv)

                un = {}
                pbc = {}
                for pi in range(2):
                    r0 = 32 * pi
                    pool = pt if pi == 0 else pu
                    pbc[pi] = pool.tile([128, Q], f32, name=f"pbc{pi}",
                                        tag=("t" if pi == 0 else "u"))
                    nc.tensor.matmul(pbc[pi], lhsT=W["e4p"][r0:r0 + 4, :],
                                     rhs=rinvb[r0:r0 + 4, :],
                                     tile_position=(r0, 0),
                                     skip_group_check=True)
                for pi in range(2):
                    u_n = wk.tile([128, Q], bf16, tag=f"un{pi}")
                    nc.vector.tensor_tensor(u_n, u2[:, pi, :], pbc[pi], OP.mult)
                    un[pi] = u_n

                # ---------- Q2 and logits ----------
                pq2 = pproj.tile([128, Q], f32, tag="proj")
                nc.tensor.matmul(pq2, lhsT=W["wouta"], rhs=un[0],
                                 start=True, stop=False)
                nc.tensor.matmul(pq2, lhsT=W["woutb"], rhs=un[1],
                                 start=False, stop=True)
                q2t = wk.tile([128, Q], bf16, tag="q2t")
                nc.scalar.copy(q2t, pq2)

                # ---------- final stage (bf16) ----------
                s1 = big.tile([128, 4, N], bf16, tag="s1")
                for ii in range(2):
                    pl = pss.tile([128, 2, N], f32, tag="sc")
                    for i2 in range(2):
                        i = 2 * ii + i2
                        nc.tensor.matmul(
                            pl[:, i2, :],
                            lhsT=q2t[:, 128 * i:128 * (i + 1)], rhs=k2t)
                    th = big.tile([128, 2, N], bf16, tag="th")
                    nc.scalar.activation(th, pl, AF.Tanh)
                    nc.gpsimd.tensor_tensor(
                        s1[:, 2 * ii:2 * ii + 2, :], th,
                        mkn_t[:, 2 * ii:2 * ii + 2, :], OP.add)

                sacc = sm.tile([128, 4], f32, tag="sacc")
                es2 = big.tile([128, 4, N], bf16, tag="es2")
                for i in range(4):
                    nc.scalar.activation(es2[:, i, :], s1[:, i, :], AF.Exp,
                                         scale=10.0, accum_out=sacc[:, i:i + 1])
                saccb = sm.tile([128, 4], bf16, tag="saccb")
                nc.vector.tensor_copy(saccb, sacc)
                pz2 = pt.tile([32, 4], f32, tag="t")
                nc.tensor.matmul(pz2, lhsT=W["p4sel"], rhs=saccb)
                z2 = sm.tile([32, 4], f32, tag="z2")
                nc.vector.tensor_copy(z2, pz2)
                # lnZ = (c2*m^2 + c1*m)*ln2 + (e - 127 + c0)*ln2
                zi = z2.bitcast(i32)
                ei = sm.tile([32, 4], i32, tag="ei")
                nc.vector.tensor_scalar(ei, zi, 23, None,
                                        OP.logical_shift_right)
                ef = sm.tile([32, 4], f32, tag="ef")
                nc.vector.tensor_copy(ef, ei)
                mi = sm.tile([32, 4], i32, tag="mi")
                nc.vector.tensor_scalar(mi, zi, 0x7FFFFF, 0x3F800000,
                                        OP.bitwise_and, OP.bitwise_or)
                mf = mi.bitcast(f32)
                acc = sm.tile([32, 4], f32, tag="lnacc")
                nc.vector.tensor_scalar(acc, mf, L2C2, L2C1, OP.mult, OP.add)
                nc.vector.tensor_tensor(acc, acc, mf, OP.mult)
                nc.vector.tensor_scalar(ef, ef, LN2, (L2C0 - 127.0) * LN2,
                                        OP.mult, OP.add)
                lnzb = sm.tile([32, 4], bf16, tag="lnzb")
                nc.vector.scalar_tensor_tensor(lnzb, acc, LN2, ef,
                                               OP.mult, OP.add)
                pbias = pt.tile([128, 4], f32, tag="t")
                nc.tensor.matmul(pbias, lhsT=W["p432"], rhs=lnzb)
                bias = sm.tile([128, 4], f32, tag="bias")
                nc.vector.tensor_copy(bias, pbias)

                out_sb = big.tile([128, 4, N], bf16, tag="outsb")
                for i in range(4):
                    nc.vector.tensor_scalar(
                        out_sb[:, i, :], s1[:, i, :], 10.0,
                        bias[:, i:i + 1], OP.mult, OP.subtract)
                nc.sync.dma_start(out=out[:, b, :, :], in_=out_sb)

    nc.compile()
    return nc


_CACHED = None


def _get_nc():
    global _CACHED
    if _CACHED is None:
        _CACHED = build_kernel()
    return _CACHED


def kernel(**inputs):
    from concourse.bass_utils import run_bass_kernel_spmd

    core_ins = _host_prep(inputs)
    nc = _get_nc()
    res = run_bass_kernel_spmd(nc, core_ins, core_ids=list(range(NCORES)))
    outs = [_unscramble(r["out"]) for r in res.results]   # each [T, NB, 2048]
    return np.concatenate(outs, axis=1)                   # [T, B, 2048]


def _unscramble(dev):
    """Device [128 q'=(t',c), nb, 4 i, 512 n] -> [T, nb, C*N] with t=32i+t'."""
    nb = dev.shape[1]
    return (dev.astype(np.float32)
            .reshape(32, C, nb, 4, N)
            .transpose(3, 0, 2, 1, 4)
            .reshape(T, nb, C * N))


# revision 25
# speedup vs baseline: 1.2922x; 1.1312x over previous
"""Trainium2 Bass kernel for nn_DecoderCell (B=128,N=512,C=4,T=128,D=128,H=8).

Strategy: pure data-parallel over batch B across 8 NeuronCores (16 b/core).
Per batch, attention scores are computed transposed ([n, q] layout, q=(t,c))
with per-head K=16 matmuls packed 4-at-a-time onto PE row groups (4-way
row-tile concurrency); the boolean mask is applied multiplicatively to the
exp'd scores (es *= (1-m)) on VectorE/GpSimd, keeping the TensorE free of
mask matmuls; softmax runs unnormalized (exp on ScalarE, denominators via an
augmented-V ones column). Final stage is bf16: tanh -> mask-add (-1e8) ->
exp/accum -> fused (x*10 - lnZ) tensor_scalar; output written bf16 and
upcast on host. Q_fixed@Wq_fixed is folded on host into a rank-2 matmul.
"""
import numpy as np
import ml_dtypes

D = 128
N = 512
C = 4
T = 128
Q = T * C          # 512 queries per batch, q = t*C + c
H = 8
DH = 16
NB = 16            # batches per core
NCORES = 8
MNEG = -1.0e8      # final-stage mask value (x10 ~= -1e9), bf16

# log2(m) on [1,2), deg-2 LSQ (max err 9.1e-3 in log2)
L2C2 = -0.33688185
L2C1 = 1.99490327
L2C0 = -1.64899236
LN2 = 0.6931471805599453

HA = [0, 1, 2, 3]
HB = [4, 5, 6, 7]
BF = ml_dtypes.bfloat16


def _perm_cols(W, heads):
    """Columns of W[*,128] so head g sits at cols 32g..32g+15, zeros after."""
    out = np.zeros_like(W)
    for g, h in enumerate(heads):
        out[:, 32 * g:32 * g + 16] = W[:, 16 * h:16 * h + 16]
    return out


def _perm_rows(W, heads):
    out = np.zeros_like(W)
    for g, h in enumerate(heads):
        out[32 * g:32 * g + 16, :] = W[16 * h:16 * h + 16, :]
    return out


def _host_prep(inputs):
    """Full-input numpy prep -> per-core input dicts."""
    ne = np.ascontiguousarray(inputs["node_embeddings"], np.float32)
    ge = np.ascontiguousarray(inputs["graph_embedding"], np.float32)
    sc = np.ascontiguousarray(inputs["step_context"], np.float32)
    mask = np.asarray(inputs["mask"])
    B = ne.shape[0]

    net = ne.transpose(0, 2, 1).astype(BF)                        # [B,D,N]
    scq = sc[:, :, :, 0, :].transpose(1, 3, 0, 2).reshape(B, D + 1, Q)
    scm = scq[:, :D, :].astype(BF)
    # [scl; ones] rank-2 rhs rows
    scl2 = np.concatenate(
        [scq[:, D:, :], np.ones((B, 1, Q), np.float32)], axis=1).astype(BF)
    m4 = mask[:, :, :, 0, :]                                      # [T,B,C,N] bool
    # keep-mask (1-m), transposed [B,N,Q] -> [B,128,4(j),Q] bf16
    mt = m4.transpose(1, 3, 0, 2).reshape(B, N, Q)
    mkeep = ((1.0 - mt.reshape(B, 4, 128, Q).transpose(0, 2, 1, 3)
              .astype(np.float32))).astype(BF)                    # [B,128,4,Q]
    # natural mask [B,Q,N] -> [B,128,4(i),N] bf16, value MNEG
    mn = m4.transpose(1, 0, 2, 3).reshape(B, Q, N)
    mknegn = (mn.reshape(B, 4, 128, N).transpose(0, 2, 1, 3).astype(np.float32)
              * np.float32(MNEG)).astype(BF)                      # [B,128,4,N]

    s = np.float32(1.0 / np.sqrt(DH))
    Wk1 = np.asarray(inputs["Wk1"], np.float32)
    Wqs = np.asarray(inputs["Wq_step"], np.float32) * s
    Wqf = np.asarray(inputs["Wq_fixed"], np.float32) * s
    wqsa, wqsb = _perm_cols(Wqs, HA), _perm_cols(Wqs, HB)
    wqfa, wqfb = _perm_cols(Wqf, HA), _perm_cols(Wqf, HB)
    # host-side Q_fixed projections (tiny): [B, 128] packed per pass
    qfa = ge @ wqfa
    qfb = ge @ wqfb
    # rank-2 lhsT per batch: rows [wq_step_last; qfix]
    qsl2a = np.stack([np.broadcast_to(wqsa[D], (B, 128)), qfa],
                     axis=1).astype(BF)                           # [B,2,128]
    qsl2b = np.stack([np.broadcast_to(wqsb[D], (B, 128)), qfb],
                     axis=1).astype(BF)
    bfw = lambda x: np.ascontiguousarray(x).astype(BF)
    # e4 padded to [128,128]: pass-A selector rows 0-3, pass-B rows 32-35
    e4p = np.zeros((128, 128), np.float32)
    for g in range(4):
        e4p[g, 32 * g:32 * g + 16] = 1.0
        e4p[32 + g, 32 * g:32 * g + 16] = 1.0
    weights = {
        "wk1a": bfw(_perm_cols(Wk1, HA)), "wk1b": bfw(_perm_cols(Wk1, HB)),
        "wqsam": bfw(wqsa[:D]), "wqsbm": bfw(wqsb[:D]),
        "wv": bfw(inputs["Wv"]),
        "wk2s": bfw(np.asarray(inputs["Wk2"], np.float32)
                    / np.float32(np.sqrt(D))),
        "wouta": bfw(_perm_rows(np.asarray(inputs["Wout"], np.float32), HA)),
        "woutb": bfw(_perm_rows(np.asarray(inputs["Wout"], np.float32), HB)),
        "e4p": e4p.astype(BF),
        # g16[32g+16, g] = 1 (Z row gather)
        "g16": np.stack([
            (np.arange(128) == 32 * g + 16).astype(np.float32)
            for g in range(4)], axis=1).astype(BF),
        # p4sel[q', t'] = 1 iff q'//4 == t'  (c-sum per step)
        "p4sel": np.stack([
            ((np.arange(128) // 4) == tp).astype(np.float32)
            for tp in range(32)], axis=1).astype(BF),
        # p432[t', q'] = 1 iff q'//4 == t'  (lnZ partition broadcast)
        "p432": np.stack([
            ((np.arange(128) // 4) == tp).astype(np.float32)
            for tp in range(32)]).astype(BF),
    }

    core_ins = []
    for ci in range(NCORES):
        b0 = ci * NB
        sl = slice(b0, b0 + NB)
        m = dict(weights)
        m.update({
            "net": np.ascontiguousarray(net[sl]),
            "scm": np.ascontiguousarray(scm[sl]),
            "scl2": np.ascontiguousarray(scl2[sl]),
            "qsl2a": np.ascontiguousarray(qsl2a[sl]),
            "qsl2b": np.ascontiguousarray(qsl2b[sl]),
            "mkeep": np.ascontiguousarray(mkeep[sl]),
            "mknegn": np.ascontiguousarray(mknegn[sl]),
        })
        core_ins.append(m)
    return core_ins


def build_kernel(nb=NB):
    import concourse.bacc as bacc
    import concourse.mybir as mybir
    import concourse.tile as tile

    dt = mybir.dt
    f32, bf16, i32 = dt.float32, dt.bfloat16, dt.int32
    AF = mybir.ActivationFunctionType
    OP = mybir.AluOpType

    nc = bacc.Bacc("TRN2", target_bir_lowering=False, debug=False,
                   num_devices=NCORES)

    din = {}
    def dram(name, shape, dtype, kind="ExternalInput"):
        din[name] = nc.dram_tensor(name, shape, dtype, kind=kind)
        return din[name]

    net = dram("net", [nb, D, N], bf16)
    scm = dram("scm", [nb, D, Q], bf16)
    scl2 = dram("scl2", [nb, 2, Q], bf16)
    qsl2a = dram("qsl2a", [nb, 2, 128], bf16)
    qsl2b = dram("qsl2b", [nb, 2, 128], bf16)
    mkeep = dram("mkeep", [nb, 128, 4, Q], bf16)
    mknegn = dram("mknegn", [nb, 128, 4, N], bf16)
    for w in ("wk1a", "wk1b", "wqsam", "wqsbm", "wv", "wk2s",
              "wouta", "woutb", "e4p", "g16", "p4sel", "p432"):
        shape = ([128, 4] if w == "g16" else
                 ([128, 32] if w == "p4sel" else
                  ([32, 128] if w == "p432" else [128, 128])))
        dram(w, shape, bf16)
    # device layout [q'=(t', c), b, i, n]; host reassembles t = 32*i + t'
    out = dram("out", [128, nb, 4, N], bf16, kind="ExternalOutput")

    with tile.TileContext(nc) as tc:
        from contextlib import ExitStack
        with ExitStack() as ctx:
            wp = ctx.enter_context(tc.tile_pool(name="wp", bufs=1))
            io = ctx.enter_context(tc.tile_pool(name="io", bufs=2))
            wk = ctx.enter_context(tc.tile_pool(name="wk", bufs=2))
            big = ctx.enter_context(tc.tile_pool(name="big", bufs=2))
            sm = ctx.enter_context(tc.tile_pool(name="sm", bufs=2))
            pss = ctx.enter_context(tc.tile_pool(name="pss", bufs=2, space="PSUM"))
            pu = ctx.enter_context(tc.tile_pool(name="pu", bufs=1, space="PSUM"))
            pproj = ctx.enter_context(tc.tile_pool(name="pproj", bufs=2, space="PSUM"))
            pt = ctx.enter_context(tc.tile_pool(name="pt", bufs=1, space="PSUM"))

            # --- static weights/constants to SBUF ---
            W = {}
            for wn in ("wk1a", "wk1b", "wqsam", "wqsbm", "wv", "wk2s",
                       "wouta", "woutb", "e4p", "g16", "p4sel", "p432"):
                t = wp.tile(list(din[wn].shape), din[wn].dtype, tag=f"w_{wn}")
                nc.sync.dma_start(out=t, in_=din[wn][:, :])
                W[wn] = t

            for b in range(nb):
                # ---------- DMA loads ----------
                net_t = io.tile([D, N], bf16, tag="net")
                nc.sync.dma_start(out=net_t, in_=net[b])
                scm_t = io.tile([D, Q], bf16, tag="scm")
                nc.sync.dma_start(out=scm_t, in_=scm[b])
                scl2_t = io.tile([2, Q], bf16, tag="scl2")
                nc.sync.dma_start(out=scl2_t, in_=scl2[b])
                q2a_t = io.tile([2, 128], bf16, tag="qsl2a")
                nc.sync.dma_start(out=q2a_t, in_=qsl2a[b])
                q2b_t = io.tile([2, 128], bf16, tag="qsl2b")
                nc.sync.dma_start(out=q2b_t, in_=qsl2b[b])
                mk_t = io.tile([128, 4, Q], bf16, tag="mkeep")
                nc.sync.dma_start(out=mk_t, in_=mkeep[b])
                mkn_t = io.tile([128, 4, N], bf16, tag="mknegn")
                nc.sync.dma_start(out=mkn_t, in_=mknegn[b])

                # ---------- projections ----------
                def proj_to_sbuf(wtile, rhs, tag):
                    ps = pproj.tile([128, N], f32, tag="proj")
                    nc.tensor.matmul(ps, lhsT=wtile, rhs=rhs)
                    sb = wk.tile([128, N], bf16, tag=tag)
                    nc.vector.tensor_copy(sb, ps)
                    return sb

                k1ta = proj_to_sbuf(W["wk1a"], net_t, "k1ta")
                k1tb = proj_to_sbuf(W["wk1b"], net_t, "k1tb")
                k2t = proj_to_sbuf(W["wk2s"], net_t, "k2t")

                # V natural per n-chunk, packed into V_aug [128, 4(j), 128]
                vauga = wk.tile([128, 4, 128], bf16, tag="vauga")
                vaugb = wk.tile([128, 4, 128], bf16, tag="vaugb")
                for va in (vauga, vaugb):
                    nc.gpsimd.memset(va, 0.0)
                    nc.gpsimd.memset(
                        va.rearrange("p j (g r) -> p j g r", g=4)[:, :, :, 16:17],
                        1.0)
                for j in range(4):
                    pv = pproj.tile([128, N], f32, tag="proj")
                    nc.tensor.matmul(
                        pv[:, :128], lhsT=net_t[:, 128 * j:128 * (j + 1)],
                        rhs=W["wv"])
                    for va, c0 in ((vauga, 0), (vaugb, 64)):
                        nc.vector.tensor_copy(
                            va[:, j, :].rearrange("p (g r) -> p g r", g=4)[:, :, 0:16],
                            pv[:, c0:c0 + 64].rearrange("p (g r) -> p g r", g=4))

                # ---------- Q1T (passes A and B) ----------
                def q1t(wm, ql2, tag):
                    ps = pproj.tile([128, Q], f32, tag="proj")
                    nc.tensor.matmul(ps, lhsT=wm, rhs=scm_t, start=True, stop=False)
                    nc.tensor.matmul(ps, lhsT=ql2, rhs=scl2_t,
                                     start=False, stop=True)
                    sb = wk.tile([128, Q], bf16, tag=tag)
                    nc.vector.tensor_copy(sb, ps)
                    return sb

                q1ta = q1t(W["wqsam"], q2a_t, "q1ta")
                q1tb = q1t(W["wqsbm"], q2b_t, "q1tb")

                # ---------- attention passes ----------
                u2 = wk.tile([128, 2, Q], bf16, tag="u2")
                pzAB = pt.tile([36, Q], f32, tag="t")
                for pi, (k1t, q1t_sb, vaug) in enumerate(
                        ((k1ta, q1ta, vauga), (k1tb, q1tb, vaugb))):
                    psu = pu.tile([128, Q], f32, tag="u")
                    for j in range(4):
                        psA = pss.tile([128, 2, Q], f32, tag="sc")
                        psB = pss.tile([128, 2, Q], f32, tag="sc")
                        # 4 score MMs on distinct row groups (concurrent)
                        for g in range(4):
                            ps2 = psA if g < 2 else psB
                            sl = slice(32 * g, 32 * g + 16)
                            nc.tensor.matmul(
                                ps2[:, g % 2, :],
                                lhsT=k1t[sl, 128 * j:128 * (j + 1)],
                                rhs=q1t_sb[sl, :], start=True, stop=True,
                                tile_position=(32 * g, 0),
                                skip_group_check=True)
                        esA = big.tile([128, 2, Q], bf16, tag="es")
                        esB = big.tile([128, 2, Q], bf16, tag="es")
                        nc.scalar.activation(esA, psA, AF.Exp)
                        nc.scalar.activation(esB, psB, AF.Exp)
                        # mask multiply (1-m): DVE / GpSimd split
                        emA = big.tile([128, 2, Q], bf16, tag="esm")
                        emB = big.tile([128, 2, Q], bf16, tag="esm")
                        engA = nc.vector if j % 2 == 0 else nc.gpsimd
                        engB = nc.gpsimd if j % 2 == 0 else nc.vector
                        for gg in range(2):
                            engA.tensor_tensor(
                                emA[:, gg, :], esA[:, gg, :], mk_t[:, j, :],
                                OP.mult)
                            engB.tensor_tensor(
                                emB[:, gg, :], esB[:, gg, :], mk_t[:, j, :],
                                OP.mult)
                        for g in range(4):
                            em = emA if g < 2 else emB
                            nc.tensor.matmul(
                                psu[32 * g:32 * g + 32, :],
                                lhsT=vaug[:, j, 32 * g:32 * g + 32],
                                rhs=em[:, g % 2, :],
                                start=(j == 0), stop=(j == 3),
                                tile_position=(0, 32 * g),
                                skip_group_check=True)
                    # copy U to sbuf; gather Z rows (pass A -> parts 0-3,
                    # pass B -> parts 32-35 via column tiling)
                    nc.vector.tensor_copy(u2[:, pi, :], psu)
                    nc.tensor.matmul(pzAB[32 * pi:32 * pi + 4, :],
                                     lhsT=W["g16"], rhs=u2[:, pi, :],
                                     tile_position=(0, 32 * pi),
                                     skip_group_check=True)

                zsb = sm.tile([36, Q], f32, tag="zsb")
                nc.vector.tensor_copy(zsb, pzAB)
                rinv = sm.tile([36, Q], f32, tag="rinv")
                nc.vector.reciprocal_approx_fast(out=rinv, in_=zsb)
                rinvb = sm.tile([36, Q], bf16, tag="rinvb")
                nc.vector.tensor_copy(rinvb, rinv)

                un = {}
                pbc = {}
                for pi in range(2):
                    r0 = 32 * pi
                    pool = pt if pi == 0 else pu
                    pbc[pi] = pool.tile([128, Q], f32, name=f"pbc{pi}",
                                        tag=("t" if pi == 0 else "u"))
                    nc.tensor.matmul(pbc[pi], lhsT=W["e4p"][r0:r0 + 4, :],
                                     rhs=rinvb[r0:r0 + 4, :],
                                     tile_position=(r0, 0),
                                     skip_group_check=True)
                for pi in range(2):
                    u_n = wk.tile([128, Q], bf16, tag=f"un{pi}")
                    nc.vector.tensor_tensor(u_n, u2[:, pi, :], pbc[pi], OP.mult)
                    un[pi] = u_n

                # ---------- Q2 and logits ----------
                pq2 = pproj.tile([128, Q], f32, tag="proj")
                nc.tensor.matmul(pq2, lhsT=W["wouta"], rhs=un[0],
                                 start=True, stop=False)
                nc.tensor.matmul(pq2, lhsT=W["woutb"], rhs=un[1],
                                 start=False, stop=True)
                q2t = wk.tile([128, Q], bf16, tag="q2t")
                nc.vector.tensor_copy(q2t, pq2)

                # ---------- final stage (bf16) ----------
                s1 = big.tile([128, 4, N], bf16, tag="s1")
                for ii in range(2):
                    pl = pss.tile([128, 2, N], f32, tag="sc")
                    for i2 in range(2):
                        i = 2 * ii + i2
                        nc.tensor.matmul(
                            pl[:, i2, :],
                            lhsT=q2t[:, 128 * i:128 * (i + 1)], rhs=k2t)
                    th = big.tile([128, 2, N], bf16, tag="th")
                    nc.scalar.activation(th, pl, AF.Tanh)
                    nc.vector.tensor_tensor(
                        s1[:, 2 * ii:2 * ii + 2, :], th,
                        mkn_t[:, 2 * ii:2 * ii + 2, :], OP.add)

                sacc = sm.tile([128, 4], f32, tag="sacc")
                es2 = big.tile([128, 4, N], bf16, tag="es2")
                for i in range(4):
                    nc.scalar.activation(es2[:, i, :], s1[:, i, :], AF.Exp,
                                         scale=10.0, accum_out=sacc[:, i:i + 1])
                saccb = sm.tile([128, 4], bf16, tag="saccb")
                nc.vector.tensor_copy(saccb, sacc)
                pz2 = pt.tile([32, 4], f32, tag="t")
                nc.tensor.matmul(pz2, lhsT=W["p4sel"], rhs=saccb)
                z2 = sm.tile([32, 4], f32, tag="z2")
                nc.vector.tensor_copy(z2, pz2)
                # lnZ = (c2*m^2 + c1*m)*ln2 + (e - 127 + c0)*ln2
                zi = z2.bitcast(i32)
                ei = sm.tile([32, 4], i32, tag="ei")
                nc.vector.tensor_scalar(ei, zi, 23, None,
                                        OP.logical_shift_right)
                ef = sm.tile([32, 4], f32, tag="ef")
                nc.vector.tensor_copy(ef, ei)
                mi = sm.tile([32, 4], i32, tag="mi")
                nc.vector.tensor_scalar(mi, zi, 0x7FFFFF, 0x3F800000,
                                        OP.bitwise_and, OP.bitwise_or)
                mf = mi.bitcast(f32)
                acc = sm.tile([32, 4], f32, tag="lnacc")
                nc.vector.tensor_scalar(acc, mf, L2C2, L2C1, OP.mult, OP.add)
                nc.vector.tensor_tensor(acc, acc, mf, OP.mult)
                nc.vector.tensor_scalar(ef, ef, LN2, (L2C0 - 127.0) * LN2,
                                        OP.mult, OP.add)
                lnzb = sm.tile([32, 4], bf16, tag="lnzb")
                nc.vector.scalar_tensor_tensor(lnzb, acc, LN2, ef,
                                               OP.mult, OP.add)
                pbias = pt.tile([128, 4], f32, tag="t")
                nc.tensor.matmul(pbias, lhsT=W["p432"], rhs=lnzb)
                bias = sm.tile([128, 4], f32, tag="bias")
                nc.vector.tensor_copy(bias, pbias)

                out_sb = big.tile([128, 4, N], bf16, tag="outsb")
                for i in range(4):
                    nc.vector.tensor_scalar(
                        out_sb[:, i, :], s1[:, i, :], 10.0,
                        bias[:, i:i + 1], OP.mult, OP.subtract)
                nc.sync.dma_start(out=out[:, b, :, :], in_=out_sb)

    nc.compile()
    return nc


_CACHED = None


def _get_nc():
    global _CACHED
    if _CACHED is None:
        _CACHED = build_kernel()
    return _CACHED


def kernel(**inputs):
    from concourse.bass_utils import run_bass_kernel_spmd

    core_ins = _host_prep(inputs)
    nc = _get_nc()
    res = run_bass_kernel_spmd(nc, core_ins, core_ids=list(range(NCORES)))
    outs = [_unscramble(r["out"]) for r in res.results]   # each [T, NB, 2048]
    return np.concatenate(outs, axis=1)                   # [T, B, 2048]


def _unscramble(dev):
    """Device [128 q'=(t',c), nb, 4 i, 512 n] -> [T, nb, C*N] with t=32i+t'."""
    nb = dev.shape[1]
    return (dev.astype(np.float32)
            .reshape(32, C, nb, 4, N)
            .transpose(3, 0, 2, 1, 4)
            .reshape(T, nb, C * N))
